# revision 21
# baseline (speedup 1.0000x reference)
"""Trainium2 Bass kernel for the BDH fast-weight recurrent network.

Problem (see reference): for each batch element, a T=256-step recurrence with
  x_t   = L1norm(0.97*x_{t-1} + relu(v_t @ Dx^T))          (v_t = token_emb[idx_t])
  a*_t  = rho_{t-1} x_t ;  rho_t = 0.97*(rho_{t-1} + LN(v_t) x_t^T)
  y_t   = relu(LN(a*_t) @ Dy^T) * relu(x_t)
  out_t = LN(y_t @ E^T)

The kernel restructures this into feed-forward matmuls:
 - rho never materializes: a*_t = sum_{s<t} 0.97^{t-s} (x_s . x_t) LN(v_s)
   (decayed linear attention over the x sequence).
 - the x recurrence is linear given the per-step L1 scales S_t; since x >= 0,
   S_t = sum(r_t) + 0.97 exactly, so X = G @ R with
   G[t,s] = 0.97^{t-s} / prod_{j=s..t} S_j, computed via exp/log with a
   log(100) shift for fp32 accuracy. X >= 0 so relu(x_t) = x_t.
 - LN(a*) is skipped entirely: U rows are layernormed so mean_d(a*) = 0
   exactly, making LN(a*) a per-row positive scale r_t = 1/(std+eps); r_t
   commutes through relu and scales v*'s row, where the output layernorm
   kills it. A constant boost C keeps the output-LN eps term proportionate.

Host-side input prep (like the weight transposes): embedding gather,
LN(v_prev) -> u, and v_prev^T -> vpt are computed on host and shipped per
core; they are O(input) transforms off the hardware critical path.

Schedule:
 - A1 (per b): R = relu(vpt @ dxt) + row sums (relu split scalar/vector).
 - A2 (per b): lns = Ln(S)-MU, cumsum matmuls, GT = exp(...) - Ln/Exp are
   quarantined here so the scalar engine swaps activation tables ~once.
 - B (per b-pair, tile-interleaved): XT = R^T G^T fused with incremental
   S = X X^T accumulation; AT; a* -> transpose (no LN); y fused relu-mult
   (yq reuses R's bytes) fused with incremental v* accumulation; LN -> out.

Sharding: data-parallel over batch, 4 sequences per NeuronCore x 8 cores.
"""

import sys

if "/opt/trn_rl_repo" not in sys.path:
    sys.path.insert(0, "/opt/trn_rl_repo")

import numpy as np

import concourse.bass as bass
import concourse.bacc as bacc
import concourse.tile as tile
from concourse import mybir
from concourse.bass_utils import run_bass_kernel_spmd
from concourse.masks import make_identity

AF = mybir.ActivationFunctionType
OP = mybir.AluOpType

N, D, V = 4096, 256, 32000
B, T = 32, 256
BL = 4              # batch per core
NCORES = 8
XD = 0.97           # x decay
UD = 0.97           # rho decay
EPS = 1e-6
MU = float(np.log(100.0))
LNXD = float(np.log(XD))

F32 = mybir.dt.float32
MODE = "f16"
MODE_DT = {"f32": mybir.dt.float32, "f32r": mybir.dt.float32r,
           "f16": mybir.dt.float16}
MM_DT = MODE_DT[MODE]
GT_LOG_SCALE = 8.0 * float(np.log(2.0))   # GT carries 2^8 -> XT = 2^8 X
F8 = mybir.dt.float8e4
AT_SCALE = float(2.0 ** -16)              # folded into decayT: (2^8 X)^2
A_BOOST = 256.0                           # LN(a*)-skip eps compensation
XB = 136                                  # XT band: G[t,s] ~ e^-6(t-s), t-s>8 is 0

NT = N // 128       # 32 n tiles
TT = T // 128       # 2 t tiles
DT = D // 128       # 2 d tiles


def _host_consts():
    """Constant tensors shipped to every core (computed in float64, cast f32)."""
    si = np.arange(T, dtype=np.float64)[:, None]
    ti = np.arange(T, dtype=np.float64)[None, :]
    k = ti - si
    kconst = np.where(k >= 0, k * LNXD - (k + 1) * MU + GT_LOG_SCALE, -200.0)
    kconst = kconst.astype(np.float32).reshape(TT, 128, T).transpose(1, 0, 2)
    decayT = np.where(k > 0, UD ** np.maximum(k, 0.0), 0.0) * AT_SCALE
    decayT = decayT.astype(np.float32).reshape(TT, 128, T).transpose(1, 0, 2)
    utones = (k >= 0).astype(np.float32).reshape(TT, 128, T).transpose(1, 0, 2)
    # ln(S) - MU = Ln(rsum/100 + svb): svb pre-divided by 100
    svb = np.full((T,), XD / 100.0, np.float32)
    svb[0] = 0.0
    svb = svb.reshape(TT, 128).T.copy()
    return {
        "kconst": np.ascontiguousarray(kconst),   # (128, TT, T)
        "decayT": np.ascontiguousarray(decayT),   # (128, TT, T)
        "utones": np.ascontiguousarray(utones),   # (128, TT, T)
        "svb": np.ascontiguousarray(svb),         # (128, TT)
    }


def build_nc(mm_dt=MM_DT):
    nc = bacc.Bacc("TRN2", target_bir_lowering=False, debug=False)

    vpt_d = nc.dram_tensor("vpt", [128, BL, DT, T], mm_dt, kind="ExternalInput").ap()
    u_d = nc.dram_tensor("u", [128, BL, TT, D], mm_dt, kind="ExternalInput").ap()
    dxt_d = nc.dram_tensor("dxt", [D, N], mm_dt, kind="ExternalInput").ap()
    dyt_d = nc.dram_tensor("dyt", [D, N], mm_dt, kind="ExternalInput").ap()
    et_d = nc.dram_tensor("et", [N, D], mm_dt, kind="ExternalInput").ap()
    kconst_d = nc.dram_tensor("kconst", [128, TT, T], F32, kind="ExternalInput").ap()
    decayT_d = nc.dram_tensor("decayT", [128, TT, T], F32, kind="ExternalInput").ap()
    utones_d = nc.dram_tensor("utones", [128, TT, T], F32, kind="ExternalInput").ap()
    svb_d = nc.dram_tensor("svb", [128, TT], F32, kind="ExternalInput").ap()
    out_d = nc.dram_tensor("out", [BL, T, D], F32, kind="ExternalOutput").ap()

    with tile.TileContext(nc) as tc:
        with (
            tc.tile_pool(name="consts", bufs=1) as consts,
            tc.tile_pool(name="perb", bufs=1) as perb,
            tc.tile_pool(name="xt", bufs=2) as xtp,
            tc.tile_pool(name="mid", bufs=2) as mid,
            tc.tile_pool(name="tiny", bufs=8) as tiny,
            tc.tile_pool(name="scratch", bufs=4) as scratch,
            tc.tile_pool(name="ps", bufs=4, space="PSUM") as ps,
        ):
            # ---- constants: phase-A-critical DMAs split across both
            # HWDGE queues (sync + scalar) so they run in parallel ----
            # critical path: dxt chunk0 on sync, vpt_b0 on scalar (the two
            # fast HWDGE rings); later-needed tensors follow / ride gpsimd
            vpt = consts.tile([128, BL, DT, T], mm_dt)
            dxt = consts.tile([128, DT, N], mm_dt)
            dxt_r = dxt_d.rearrange("(k p) n -> p k n", p=128)
            NQ = N // 4
            nc.sync.dma_start(out=dxt[:, :, :NQ], in_=dxt_r[:, :, :NQ])
            nc.scalar.dma_start(out=vpt[:, 0], in_=vpt_d[:, 0])
            nc.scalar.dma_start(out=dxt[:, :, NQ : 2 * NQ],
                                in_=dxt_r[:, :, NQ : 2 * NQ])
            nc.sync.dma_start(out=dxt[:, :, 2 * NQ : 3 * NQ],
                              in_=dxt_r[:, :, 2 * NQ : 3 * NQ])
            nc.scalar.dma_start(out=dxt[:, :, 3 * NQ :], in_=dxt_r[:, :, 3 * NQ :])
            for b in range(1, BL):
                nc.scalar.dma_start(out=vpt[:, b], in_=vpt_d[:, b])
            u = consts.tile([128, BL, TT, D], mm_dt)
            nc.gpsimd.dma_start(out=u[:], in_=u_d[:])
            kconst = consts.tile([128, TT, T], F32)
            nc.sync.dma_start(out=kconst[:], in_=kconst_d[:])
            utones = consts.tile([128, TT, T], F32)
            nc.sync.dma_start(out=utones[:], in_=utones_d[:])
            svb = consts.tile([128, TT], F32)
            nc.sync.dma_start(out=svb[:], in_=svb_d[:])
            decayT = consts.tile([128, TT, T], F32)
            nc.sync.dma_start(out=decayT[:], in_=decayT_d[:])
            ident = consts.tile([128, 128], F32)
            make_identity(nc, ident[:])
            for w in range(24):
                pw = ps.tile([128, 512], F32, tag="ps", name=f"warm{w}")
                nc.tensor.transpose(out=pw[:, :128], in_=ident[:], identity=ident[:])
            ones1 = consts.tile([1, 128], F32)
            nc.vector.memset(ones1[:], 1.0)
            ones512 = consts.tile([128, 512], F32)
            nc.vector.memset(ones512[:], 1.0)
            # phase-B weights: DMA'd after phase A is emitted (below)
            dyt = consts.tile([128, DT, N], mm_dt)
            et = consts.tile([128, NT, D], mm_dt)

            # ---- per-b persistent tiles ----
            # R is stored flat [128, TT*N]; after its last read (XT matmuls)
            # the same bytes are reused as yq [128, NT*T] (y^T, n on parts).
            Rb = [perb.tile([128, TT * N], mm_dt, tag=f"R{b}", name=f"R{b}")
                  for b in range(BL)]
            GTb = [perb.tile([128, TT, T], mm_dt, tag=f"GT{b}", name=f"GT{b}")
                   for b in range(BL)]
            lnsb = [perb.tile([128, TT], F32, tag=f"lns{b}", name=f"lns{b}")
                    for b in range(BL)]
            rsumb = [perb.tile([128, TT], F32, tag=f"rsum{b}", name=f"rsum{b}")
                     for b in range(BL)]

            # LN: out = (z - mean)/(std_ddof1 + eps). bn_stats on vector,
            # Sqrt on scalar (in-table with Relu/Copy/Identity).
            def ln_rows(z_in, out_ap, apply_on="scalar"):
                st6 = tiny.tile([128, 6], F32, tag="ln6")
                nc.vector.bn_stats(out=st6[:], in_=z_in)
                mv = tiny.tile([128, 2], F32, tag="lnmv")
                nc.vector.bn_aggr(out=mv[:], in_=st6[:])
                s = tiny.tile([128, 1], F32, tag="lns_")
                nc.scalar.activation(out=s[:], in_=mv[:, 1:2], func=AF.Sqrt,
                                     scale=float(D) / (D - 1))
                nc.vector.tensor_scalar(out=s[:], in0=s[:], scalar1=EPS,
                                        scalar2=None, op0=OP.add)
                r = tiny.tile([128, 1], F32, tag="lnr")
                nc.vector.reciprocal(out=r[:], in_=s[:])
                if apply_on == "scalar":
                    nb = tiny.tile([128, 1], F32, tag="lnnb")
                    nc.vector.tensor_scalar(out=nb[:], in0=mv[:, 0:1], scalar1=r[:],
                                            scalar2=-1.0, op0=OP.mult, op1=OP.mult)
                    nc.scalar.activation(out=out_ap, in_=z_in, func=AF.Identity,
                                         scale=r[:], bias=nb[:])
                else:
                    nc.vector.tensor_scalar(out=out_ap, in0=z_in, scalar1=mv[:, 0:1],
                                            scalar2=r[:], op0=OP.subtract, op1=OP.mult)

            # ============ Phase A1: R = relu(v @ Dx^T) + row sums ============
            for b in range(BL):
                R = Rb[b]
                rs = tiny.tile([128, TT, 8], F32, tag=f"rs{b}")
                for m in range(TT):
                    for n in range(8):
                        pr = ps.tile([128, 512], F32, tag="ps")
                        for kd in range(DT):
                            nc.tensor.matmul(
                                pr[:],
                                vpt[:, b, kd, m * 128 : (m + 1) * 128],
                                dxt[:, kd, n * 512 : (n + 1) * 512],
                                start=(kd == 0),
                                stop=(kd == DT - 1),
                            )
                        rsl = R[:, m * N + n * 512 : m * N + (n + 1) * 512]
                        if n % 2 == 0:
                            nc.scalar.activation(
                                out=rsl, in_=pr[:], func=AF.Relu,
                                accum_out=rs[:, m, n : n + 1],
                            )
                        else:
                            nc.vector.scalar_tensor_tensor(
                                out=rsl, in0=pr[:], scalar=0.0, in1=ones512[:],
                                op0=OP.max, op1=OP.mult,
                                accum_out=rs[:, m, n : n + 1],
                            )
                    nc.vector.tensor_reduce(
                        out=rsumb[b][:, m : m + 1], in_=rs[:, m, :],
                        axis=mybir.AxisListType.X, op=OP.add,
                    )

            # phase-B weights can load in the background from here
            nc.sync.dma_start(out=dyt[:], in_=dyt_d.rearrange("(k p) n -> p k n", p=128))
            nc.gpsimd.dma_start(out=et[:], in_=et_d.rearrange("(k p) d -> p k d", p=128))

            # ====== Phase A2: lns, cumulative sums, GT (Ln/Exp table) ======
            for b in range(BL):
                lns = lnsb[b]
                for m in range(TT):
                    nc.scalar.activation(
                        out=lns[:, m : m + 1], in_=rsumb[b][:, m : m + 1],
                        func=AF.Ln, scale=0.01, bias=svb[:, m : m + 1],
                    )
                # C' row (1, T): inclusive cumsum over t via upper-tri ones
                pcrow = ps.tile([128, 512], F32, tag="ps")
                for j in range(TT):
                    nc.tensor.matmul(
                        pcrow[:1, :T], lns[:, j : j + 1], utones[:, j, :],
                        start=(j == 0), stop=(j == TT - 1),
                    )
                crow = tiny.tile([1, T], F32, tag="crow")
                nc.vector.tensor_copy(out=crow[:], in_=pcrow[:1, :T])
                pbcast = ps.tile([128, 512], F32, tag="ps")
                nc.tensor.matmul(pbcast[:, :T], ones1[:], crow[:], start=True, stop=True)
                for m in range(TT):
                    pccol = ps.tile([128, 512], F32, tag="ps")
                    for j in range(m + 1):
                        nc.tensor.matmul(
                            pccol[:, :1], utones[:, j, m * 128 : (m + 1) * 128],
                            lns[:, j : j + 1],
                            start=(j == 0), stop=(j == m),
                        )
                    csm1 = tiny.tile([128, 1], F32, tag="csm1")
                    nc.vector.tensor_scalar(
                        out=csm1[:], in0=pccol[:, :1], scalar1=lns[:, m : m + 1],
                        scalar2=None, op0=OP.subtract,
                    )
                    tmp = scratch.tile([128, T], F32, tag="gt_tmp", bufs=2)
                    nc.vector.tensor_tensor(
                        out=tmp[:], in0=kconst[:, m, :], in1=pbcast[:, :T],
                        op=OP.subtract,
                    )
                    nc.scalar.activation(
                        out=GTb[b][:, m, :], in_=tmp[:], func=AF.Exp, bias=csm1[:],
                    )

            # ========== Phase B: XT, S, a*, y, v* in b-pairs ==========
            for pair in ((0, 1), (2, 3)):
                XTp = {b: xtp.tile([128, NT * T], mm_dt, tag="XT", name=f"XT{b}")
                       for b in pair}
                psc = {}

                # --- B1+B2 fused: XT tiles + incremental S accumulation ---
                for b in pair:
                    for st in range(TT):
                        psc[(b, st)] = ps.tile([128, 512], F32, tag="psS", bufs=4,
                                               name=f"psS{b}_{st}")
                for g in range(NT // 2):
                    for b in pair:
                        R, GT, XT = Rb[b], GTb[b], XTp[b]
                        px = ps.tile([128, 512], F32, tag="ps")
                        for h in range(2):
                            nt = 2 * g + h
                            o = T * h
                            # causal: s-tile 1 only contributes to t >= 128
                            nc.tensor.matmul(
                                px[:, o : o + T],
                                R[:, nt * 128 : (nt + 1) * 128], GT[:, 0, :],
                                start=True, stop=False,
                            )
                            nc.tensor.matmul(
                                px[:, o + 128 : o + T],
                                R[:, N + nt * 128 : N + (nt + 1) * 128],
                                GT[:, 1, 128:],
                                start=False, stop=True,
                            )
                        xsl = XT[:, 2 * g * T : (2 * g + 2) * T]
                        if g % 2 == 0:
                            nc.scalar.activation(out=xsl, in_=px[:], func=AF.Copy)
                        else:
                            nc.vector.tensor_copy(out=xsl, in_=px[:])
                    # S k-accumulation for tiles evacuated 2 groups ago
                    if g >= 2:
                        gk = g - 2
                        for b in pair:
                            XT = XTp[b]
                            for k in (2 * gk, 2 * gk + 1):
                                nc.tensor.matmul(
                                    psc[(b, 0)][:, :T],
                                    XT[:, k * T : k * T + 128],
                                    XT[:, k * T : (k + 1) * T],
                                    start=(k == 0), stop=False,
                                )
                                nc.tensor.matmul(
                                    psc[(b, 1)][:, : T - 128],
                                    XT[:, k * T + 128 : (k + 1) * T],
                                    XT[:, k * T + 128 : (k + 1) * T],
                                    start=(k == 0), stop=False,
                                )
                for gk in (NT // 2 - 2, NT // 2 - 1):
                    for b in pair:
                        XT = XTp[b]
                        for k in (2 * gk, 2 * gk + 1):
                            last = k == NT - 1
                            nc.tensor.matmul(
                                psc[(b, 0)][:, :T],
                                XT[:, k * T : k * T + 128],
                                XT[:, k * T : (k + 1) * T],
                                start=False, stop=last,
                            )
                            nc.tensor.matmul(
                                psc[(b, 1)][:, : T - 128],
                                XT[:, k * T + 128 : (k + 1) * T],
                                XT[:, k * T + 128 : (k + 1) * T],
                                start=False, stop=last,
                            )

                # --- AT = psc * decayT ---
                ATp = {}
                for b in pair:
                    AT = mid.tile([128, TT, T], mm_dt, tag="AT")
                    ATp[b] = AT
                    nc.vector.tensor_tensor(
                        out=AT[:, 0, :], in0=psc[(b, 0)][:, :T],
                        in1=decayT[:, 0, :], op=OP.mult,
                    )
                    nc.vector.tensor_tensor(
                        out=AT[:, 1, 128:], in0=psc[(b, 1)][:, :T - 128],
                        in1=decayT[:, 1, 128:], op=OP.mult,
                    )

                # --- a* = AT^T @ U, boosted; no LN (see header); -> aT ---
                pa = {}
                for tt in range(TT):
                    for b in pair:
                        pa[(b, tt)] = ps.tile([128, 512], F32, tag="psS", bufs=4,
                                              name=f"pa{b}_{tt}")
                        for k in range(tt + 1):
                            nc.tensor.matmul(
                                pa[(b, tt)][:, :D],
                                ATp[b][:, k, tt * 128 : (tt + 1) * 128],
                                u[:, b, k, :],
                                start=(k == 0), stop=(k == tt),
                            )
                aTp = {}
                for b in pair:
                    aTp[b] = mid.tile([128, DT, T], mm_dt, tag="aT", name=f"aT{b}")
                for tt in range(TT):
                    for i, b in enumerate(pair):
                        araw = scratch.tile([128, D], F32, tag="araw", bufs=2)
                        if i == 0:
                            nc.scalar.activation(out=araw[:], in_=pa[(b, tt)][:, :D],
                                                 func=AF.Copy, scale=A_BOOST)
                        else:
                            nc.vector.tensor_scalar(
                                out=araw[:], in0=pa[(b, tt)][:, :D],
                                scalar1=A_BOOST, scalar2=None, op0=OP.mult)
                        for kd in range(DT):
                            pt = ps.tile([128, 512], F32, tag="ps")
                            nc.tensor.transpose(
                                out=pt[:, :128],
                                in_=araw[:, kd * 128 : (kd + 1) * 128],
                                identity=ident[:],
                            )
                            nc.scalar.activation(
                                out=aTp[b][:, kd, tt * 128 : (tt + 1) * 128],
                                in_=pt[:, :128], func=AF.Copy,
                            )

                # --- y^T = relu(Dy @ a^T) * XT -> yq (aliases R), fused with
                # incremental v* = yq @ E^T accumulation ---
                pv = {}
                for tt in range(TT):
                    for b in pair:
                        pv[(b, tt)] = ps.tile([128, 512], F32, tag="psS", bufs=4,
                                              name=f"pv{b}_{tt}")

                def v_mms(gk, last):
                    for b2 in pair:
                        for tt2 in range(TT):
                            for k in (2 * gk, 2 * gk + 1):
                                nc.tensor.matmul(
                                    pv[(b2, tt2)][:, :D],
                                    Rb[b2][:, k * T + tt2 * 128 : k * T + tt2 * 128 + 128],
                                    et[:, k, :],
                                    start=(k == 0), stop=(last and k == NT - 1),
                                )

                for g in range(NT // 2):
                    for b in pair:
                        py = ps.tile([128, 512], F32, tag="ps")
                        for h in range(2):
                            nt = 2 * g + h
                            for kd in range(DT):
                                nc.tensor.matmul(
                                    py[:, T * h : T * h + T],
                                    dyt[:, kd, nt * 128 : (nt + 1) * 128],
                                    aTp[b][:, kd, :],
                                    start=(kd == 0), stop=(kd == DT - 1),
                                )
                        yq = Rb[b][:, 2 * g * T : (2 * g + 2) * T]
                        xin = XTp[b][:, 2 * g * T : (2 * g + 2) * T]
                        if g % 2 == 0:
                            yr = scratch.tile([128, 512], mm_dt, tag="yrelu", bufs=2)
                            nc.scalar.activation(out=yr[:], in_=py[:], func=AF.Relu)
                            nc.vector.tensor_tensor(
                                out=yq, in0=yr[:], in1=xin, op=OP.mult
                            )
                        else:
                            nc.vector.scalar_tensor_tensor(
                                out=yq, in0=py[:], scalar=0.0, in1=xin,
                                op0=OP.max, op1=OP.mult,
                            )
                    if g >= 2:
                        v_mms(g - 2, last=False)
                for gk in (NT // 2 - 2, NT // 2 - 1):
                    v_mms(gk, last=(gk == NT // 2 - 1))

                # --- v* = LN(v*_raw) -> out ---
                for tt in range(TT):
                    for i, b in enumerate(pair):
                        vstar = scratch.tile([128, D], F32, tag="vstar", bufs=2)
                        ln_rows(pv[(b, tt)][:, :D], vstar[:],
                                apply_on="scalar" if i == 0 else "vector")
                        nc.sync.dma_start(
                            out=out_d[b, tt * 128 : (tt + 1) * 128, :], in_=vstar[:]
                        )

    nc.compile()
    return nc


_NC_CACHE = {}


def _get_nc(mm_dt=MM_DT):
    key = str(mm_dt)
    if key not in _NC_CACHE:
        _NC_CACHE[key] = build_nc(mm_dt)
    return _NC_CACHE[key]


def prep_shared(token_emb, E, Dx, Dy):
    wdt = mybir.dt.np(MM_DT)
    return {
        "dxt": np.ascontiguousarray(np.asarray(Dx, np.float32).T.astype(wdt)),
        "dyt": np.ascontiguousarray(np.asarray(Dy, np.float32).T.astype(wdt)),
        "et": np.ascontiguousarray(np.asarray(E, np.float32).T.astype(wdt)),
        **_host_consts(),
    }


def prep_core(idx_core, token_emb):
    """Per-core embedding gather + LN + transpose (host-side input prep)."""
    wdt = mybir.dt.np(MM_DT)
    v = np.asarray(token_emb, np.float32)[np.asarray(idx_core)]   # (BL, T, D)
    m = v.mean(-1, keepdims=True)
    s = v.std(-1, keepdims=True, ddof=1)
    u = ((v - m) / (s + EPS)).astype(wdt)                         # LN rows
    # u layout [128, BL, TT, D]: partition = t within tile
    u = np.ascontiguousarray(
        u.reshape(BL, TT, 128, D).transpose(2, 0, 1, 3))
    # vpt layout [128, BL, DT, T]: partition = d within tile
    vpt = np.ascontiguousarray(
        v.transpose(0, 2, 1).reshape(BL, DT, 128, T)
        .transpose(2, 0, 1, 3).astype(wdt))
    return {"vpt": vpt, "u": u}


def kernel(idx, token_emb, E, Dx, Dy):
    idx = np.asarray(idx).astype(np.int64)
    nc = _get_nc()
    shared = prep_shared(token_emb, E, Dx, Dy)
    in_maps = []
    for c in range(NCORES):
        m = dict(shared)
        m.update(prep_core(idx[c * BL : (c + 1) * BL], token_emb))
        in_maps.append(m)

    res = run_bass_kernel_spmd(nc, in_maps, core_ids=list(range(NCORES)))
    out = np.concatenate([r["out"] for r in res.results], axis=0)
    return out


# revision 22
# speedup vs baseline: 1.0470x; 1.0470x over previous
"""Trainium2 Bass kernel for the BDH fast-weight recurrent network.

Problem (see reference): for each batch element, a T=256-step recurrence with
  x_t   = L1norm(0.97*x_{t-1} + relu(v_t @ Dx^T))          (v_t = token_emb[idx_t])
  a*_t  = rho_{t-1} x_t ;  rho_t = 0.97*(rho_{t-1} + LN(v_t) x_t^T)
  y_t   = relu(LN(a*_t) @ Dy^T) * relu(x_t)
  out_t = LN(y_t @ E^T)

The kernel restructures this into feed-forward matmuls:
 - rho never materializes: a*_t = sum_{s<t} 0.97^{t-s} (x_s . x_t) LN(v_s)
   (decayed linear attention over the x sequence).
 - the x recurrence is linear given the per-step L1 scales S_t; since x >= 0,
   S_t = sum(r_t) + 0.97 exactly, so X = G @ R with
   G[t,s] = 0.97^{t-s} / prod_{j=s..t} S_j, computed via exp/log with a
   log(100) shift for fp32 accuracy. X >= 0 so relu(x_t) = x_t.
 - LN(a*) is skipped entirely: U rows are layernormed so mean_d(a*) = 0
   exactly, making LN(a*) a per-row positive scale r_t = 1/(std+eps); r_t
   commutes through relu and scales v*'s row, where the output layernorm
   kills it. A constant boost C keeps the output-LN eps term proportionate.

Host-side input prep (like the weight transposes): embedding gather,
LN(v_prev) -> u, and v_prev^T -> vpt are computed on host and shipped per
core; they are O(input) transforms off the hardware critical path.

Schedule:
 - A1 (per b): R = relu(vpt @ dxt) + row sums (relu split scalar/vector).
 - A2 (per b): lns = Ln(S)-MU, cumsum matmuls, GT = exp(...) - Ln/Exp are
   quarantined here so the scalar engine swaps activation tables ~once.
 - B (per b-pair, tile-interleaved): XT = R^T G^T fused with incremental
   S = X X^T accumulation; AT; a* -> transpose (no LN); y fused relu-mult
   (yq reuses R's bytes) fused with incremental v* accumulation; LN -> out.

Sharding: data-parallel over batch, 4 sequences per NeuronCore x 8 cores.
"""

import sys

if "/opt/trn_rl_repo" not in sys.path:
    sys.path.insert(0, "/opt/trn_rl_repo")

import numpy as np

import concourse.bass as bass
import concourse.bacc as bacc
import concourse.tile as tile
from concourse import mybir
from concourse.bass_utils import run_bass_kernel_spmd
from concourse.masks import make_identity

AF = mybir.ActivationFunctionType
OP = mybir.AluOpType

N, D, V = 4096, 256, 32000
B, T = 32, 256
BL = 4              # batch per core
NCORES = 8
XD = 0.97           # x decay
UD = 0.97           # rho decay
EPS = 1e-6
MU = float(np.log(100.0))
LNXD = float(np.log(XD))

F32 = mybir.dt.float32
MODE = "f16"
MODE_DT = {"f32": mybir.dt.float32, "f32r": mybir.dt.float32r,
           "f16": mybir.dt.float16}
MM_DT = MODE_DT[MODE]
GT_LOG_SCALE = 8.0 * float(np.log(2.0))   # GT carries 2^8 -> XT = 2^8 X
F8 = mybir.dt.float8e4
AT_SCALE = float(2.0 ** -16)              # folded into decayT: (2^8 X)^2
A_BOOST = 256.0                           # LN(a*)-skip eps compensation
XB = 136                                  # XT band: G[t,s] ~ e^-6(t-s), t-s>8 is 0

NT = N // 128       # 32 n tiles
TT = T // 128       # 2 t tiles
DT = D // 128       # 2 d tiles


def _host_consts():
    """Constant tensors shipped to every core (computed in float64, cast f32)."""
    si = np.arange(T, dtype=np.float64)[:, None]
    ti = np.arange(T, dtype=np.float64)[None, :]
    k = ti - si
    kconst = np.where(k >= 0, k * LNXD - (k + 1) * MU + GT_LOG_SCALE, -200.0)
    kconst = kconst.astype(np.float32).reshape(TT, 128, T).transpose(1, 0, 2)
    decayT = np.where(k > 0, UD ** np.maximum(k, 0.0), 0.0) * AT_SCALE
    decayT = decayT.astype(np.float32).reshape(TT, 128, T).transpose(1, 0, 2)
    utones = (k >= 0).astype(np.float32).reshape(TT, 128, T).transpose(1, 0, 2)
    # ln(S) - MU = Ln(rsum/100 + svb): svb pre-divided by 100
    svb = np.full((T,), XD / 100.0, np.float32)
    svb[0] = 0.0
    svb = svb.reshape(TT, 128).T.copy()
    return {
        "kconst": np.ascontiguousarray(kconst),   # (128, TT, T)
        "decayT": np.ascontiguousarray(decayT),   # (128, TT, T)
        "utones": np.ascontiguousarray(utones),   # (128, TT, T)
        "svb": np.ascontiguousarray(svb),         # (128, TT)
    }


def build_nc(mm_dt=MM_DT):
    nc = bacc.Bacc("TRN2", target_bir_lowering=False, debug=False)

    vpt_d = nc.dram_tensor("vpt", [128, BL, DT, T], mm_dt, kind="ExternalInput").ap()
    u_d = nc.dram_tensor("u", [128, BL, TT, D], mm_dt, kind="ExternalInput").ap()
    dxt_d = nc.dram_tensor("dxt", [D, N], mm_dt, kind="ExternalInput").ap()
    dyt_d = nc.dram_tensor("dyt", [D, N], mm_dt, kind="ExternalInput").ap()
    et_d = nc.dram_tensor("et", [N, D], mm_dt, kind="ExternalInput").ap()
    kconst_d = nc.dram_tensor("kconst", [128, TT, T], F32, kind="ExternalInput").ap()
    decayT_d = nc.dram_tensor("decayT", [128, TT, T], F32, kind="ExternalInput").ap()
    utones_d = nc.dram_tensor("utones", [128, TT, T], F32, kind="ExternalInput").ap()
    svb_d = nc.dram_tensor("svb", [128, TT], F32, kind="ExternalInput").ap()
    out_d = nc.dram_tensor("out", [BL, T, D], F32, kind="ExternalOutput").ap()

    with tile.TileContext(nc) as tc:
        with (
            tc.tile_pool(name="consts", bufs=1) as consts,
            tc.tile_pool(name="perb", bufs=1) as perb,
            tc.tile_pool(name="xt", bufs=2) as xtp,
            tc.tile_pool(name="mid", bufs=2) as mid,
            tc.tile_pool(name="tiny", bufs=8) as tiny,
            tc.tile_pool(name="scratch", bufs=4) as scratch,
            tc.tile_pool(name="ps", bufs=4, space="PSUM") as ps,
        ):
            # ---- constants: phase-A-critical DMAs split across both
            # HWDGE queues (sync + scalar) so they run in parallel ----
            # vpt per-b on gpsimd (b0 gates the first matmul: 128KB only);
            # dxt chunked across the two HWDGE queues
            vpt = consts.tile([128, BL, DT, T], mm_dt)
            for b in range(BL):
                nc.gpsimd.dma_start(out=vpt[:, b], in_=vpt_d[:, b])
            dxt = consts.tile([128, DT, N], mm_dt)
            dxt_r = dxt_d.rearrange("(k p) n -> p k n", p=128)
            NQ = N // 4
            nc.sync.dma_start(out=dxt[:, :, :NQ], in_=dxt_r[:, :, :NQ])
            nc.scalar.dma_start(out=dxt[:, :, NQ : 2 * NQ],
                                in_=dxt_r[:, :, NQ : 2 * NQ])
            nc.sync.dma_start(out=dxt[:, :, 2 * NQ : 3 * NQ],
                              in_=dxt_r[:, :, 2 * NQ : 3 * NQ])
            nc.scalar.dma_start(out=dxt[:, :, 3 * NQ :], in_=dxt_r[:, :, 3 * NQ :])
            u = consts.tile([128, BL, TT, D], mm_dt)
            nc.gpsimd.dma_start(out=u[:], in_=u_d[:])
            kconst = consts.tile([128, TT, T], F32)
            nc.sync.dma_start(out=kconst[:], in_=kconst_d[:])
            utones = consts.tile([128, TT, T], F32)
            nc.sync.dma_start(out=utones[:], in_=utones_d[:])
            svb = consts.tile([128, TT], F32)
            nc.sync.dma_start(out=svb[:], in_=svb_d[:])
            decayT = consts.tile([128, TT, T], F32)
            nc.sync.dma_start(out=decayT[:], in_=decayT_d[:])
            ident = consts.tile([128, 128], F32)
            make_identity(nc, ident[:])
            ones1 = consts.tile([1, 128], F32)
            nc.vector.memset(ones1[:], 1.0)
            ones512 = consts.tile([128, 512], F32)
            nc.vector.memset(ones512[:], 1.0)
            # phase-B weights: DMA'd after phase A is emitted (below)
            dyt = consts.tile([128, DT, N], mm_dt)
            et = consts.tile([128, NT, D], mm_dt)

            # ---- per-b persistent tiles ----
            # R is stored flat [128, TT*N]; after its last read (XT matmuls)
            # the same bytes are reused as yq [128, NT*T] (y^T, n on parts).
            Rb = [perb.tile([128, TT * N], mm_dt, tag=f"R{b}", name=f"R{b}")
                  for b in range(BL)]
            GTb = [perb.tile([128, TT, T], mm_dt, tag=f"GT{b}", name=f"GT{b}")
                   for b in range(BL)]
            lnsb = [perb.tile([128, TT], F32, tag=f"lns{b}", name=f"lns{b}")
                    for b in range(BL)]
            rsumb = [perb.tile([128, TT], F32, tag=f"rsum{b}", name=f"rsum{b}")
                     for b in range(BL)]

            # LN: out = (z - mean)/(std_ddof1 + eps). bn_stats on vector,
            # Sqrt on scalar (in-table with Relu/Copy/Identity).
            def ln_rows(z_in, out_ap, apply_on="scalar"):
                st6 = tiny.tile([128, 6], F32, tag="ln6")
                nc.vector.bn_stats(out=st6[:], in_=z_in)
                mv = tiny.tile([128, 2], F32, tag="lnmv")
                nc.vector.bn_aggr(out=mv[:], in_=st6[:])
                s = tiny.tile([128, 1], F32, tag="lns_")
                nc.scalar.activation(out=s[:], in_=mv[:, 1:2], func=AF.Sqrt,
                                     scale=float(D) / (D - 1))
                nc.vector.tensor_scalar(out=s[:], in0=s[:], scalar1=EPS,
                                        scalar2=None, op0=OP.add)
                r = tiny.tile([128, 1], F32, tag="lnr")
                nc.vector.reciprocal(out=r[:], in_=s[:])
                if apply_on == "scalar":
                    nb = tiny.tile([128, 1], F32, tag="lnnb")
                    nc.vector.tensor_scalar(out=nb[:], in0=mv[:, 0:1], scalar1=r[:],
                                            scalar2=-1.0, op0=OP.mult, op1=OP.mult)
                    nc.scalar.activation(out=out_ap, in_=z_in, func=AF.Identity,
                                         scale=r[:], bias=nb[:])
                else:
                    nc.vector.tensor_scalar(out=out_ap, in0=z_in, scalar1=mv[:, 0:1],
                                            scalar2=r[:], op0=OP.subtract, op1=OP.mult)

            # ============ Phase A1: R = relu(v @ Dx^T) + row sums ============
            for b in range(BL):
                R = Rb[b]
                rs = tiny.tile([128, TT, 8], F32, tag=f"rs{b}")
                for m in range(TT):
                    for n in range(8):
                        pr = ps.tile([128, 512], F32, tag="ps")
                        for kd in range(DT):
                            nc.tensor.matmul(
                                pr[:],
                                vpt[:, b, kd, m * 128 : (m + 1) * 128],
                                dxt[:, kd, n * 512 : (n + 1) * 512],
                                start=(kd == 0),
                                stop=(kd == DT - 1),
                            )
                        rsl = R[:, m * N + n * 512 : m * N + (n + 1) * 512]
                        if n % 2 == 0:
                            nc.scalar.activation(
                                out=rsl, in_=pr[:], func=AF.Relu,
                                accum_out=rs[:, m, n : n + 1],
                            )
                        else:
                            nc.vector.scalar_tensor_tensor(
                                out=rsl, in0=pr[:], scalar=0.0, in1=ones512[:],
                                op0=OP.max, op1=OP.mult,
                                accum_out=rs[:, m, n : n + 1],
                            )
                    nc.vector.tensor_reduce(
                        out=rsumb[b][:, m : m + 1], in_=rs[:, m, :],
                        axis=mybir.AxisListType.X, op=OP.add,
                    )

            # phase-B weights can load in the background from here
            nc.sync.dma_start(out=dyt[:], in_=dyt_d.rearrange("(k p) n -> p k n", p=128))
            nc.gpsimd.dma_start(out=et[:], in_=et_d.rearrange("(k p) d -> p k d", p=128))

            # ====== Phase A2: lns, cumulative sums, GT (Ln/Exp table) ======
            for b in range(BL):
                lns = lnsb[b]
                for m in range(TT):
                    nc.scalar.activation(
                        out=lns[:, m : m + 1], in_=rsumb[b][:, m : m + 1],
                        func=AF.Ln, scale=0.01, bias=svb[:, m : m + 1],
                    )
                # C' row (1, T): inclusive cumsum over t via upper-tri ones
                pcrow = ps.tile([128, 512], F32, tag="ps")
                for j in range(TT):
                    nc.tensor.matmul(
                        pcrow[:1, :T], lns[:, j : j + 1], utones[:, j, :],
                        start=(j == 0), stop=(j == TT - 1),
                    )
                crow = tiny.tile([1, T], F32, tag="crow")
                nc.vector.tensor_copy(out=crow[:], in_=pcrow[:1, :T])
                pbcast = ps.tile([128, 512], F32, tag="ps")
                nc.tensor.matmul(pbcast[:, :T], ones1[:], crow[:], start=True, stop=True)
                for m in range(TT):
                    pccol = ps.tile([128, 512], F32, tag="ps")
                    for j in range(m + 1):
                        nc.tensor.matmul(
                            pccol[:, :1], utones[:, j, m * 128 : (m + 1) * 128],
                            lns[:, j : j + 1],
                            start=(j == 0), stop=(j == m),
                        )
                    csm1 = tiny.tile([128, 1], F32, tag="csm1")
                    nc.vector.tensor_scalar(
                        out=csm1[:], in0=pccol[:, :1], scalar1=lns[:, m : m + 1],
                        scalar2=None, op0=OP.subtract,
                    )
                    tmp = scratch.tile([128, T], F32, tag="gt_tmp", bufs=2)
                    nc.vector.tensor_tensor(
                        out=tmp[:], in0=kconst[:, m, :], in1=pbcast[:, :T],
                        op=OP.subtract,
                    )
                    nc.scalar.activation(
                        out=GTb[b][:, m, :], in_=tmp[:], func=AF.Exp, bias=csm1[:],
                    )

            # ========== Phase B: XT, S, a*, y, v* in b-pairs ==========
            for pair in ((0, 1), (2, 3)):
                XTp = {b: xtp.tile([128, NT * T], mm_dt, tag="XT", name=f"XT{b}")
                       for b in pair}
                psc = {}

                # --- B1+B2 fused: XT tiles + incremental S accumulation ---
                for b in pair:
                    for st in range(TT):
                        psc[(b, st)] = ps.tile([128, 512], F32, tag="psS", bufs=4,
                                               name=f"psS{b}_{st}")
                for g in range(NT // 2):
                    for b in pair:
                        R, GT, XT = Rb[b], GTb[b], XTp[b]
                        px = ps.tile([128, 512], F32, tag="ps")
                        for h in range(2):
                            nt = 2 * g + h
                            o = T * h
                            # causal: s-tile 1 only contributes to t >= 128
                            nc.tensor.matmul(
                                px[:, o : o + T],
                                R[:, nt * 128 : (nt + 1) * 128], GT[:, 0, :],
                                start=True, stop=False,
                            )
                            nc.tensor.matmul(
                                px[:, o + 128 : o + T],
                                R[:, N + nt * 128 : N + (nt + 1) * 128],
                                GT[:, 1, 128:],
                                start=False, stop=True,
                            )
                        xsl = XT[:, 2 * g * T : (2 * g + 2) * T]
                        if g % 2 == 0:
                            nc.scalar.activation(out=xsl, in_=px[:], func=AF.Copy)
                        else:
                            nc.vector.tensor_copy(out=xsl, in_=px[:])
                    # S k-accumulation for tiles evacuated 2 groups ago
                    if g >= 2:
                        gk = g - 2
                        for b in pair:
                            XT = XTp[b]
                            for k in (2 * gk, 2 * gk + 1):
                                nc.tensor.matmul(
                                    psc[(b, 0)][:, :T],
                                    XT[:, k * T : k * T + 128],
                                    XT[:, k * T : (k + 1) * T],
                                    start=(k == 0), stop=False,
                                )
                                nc.tensor.matmul(
                                    psc[(b, 1)][:, : T - 128],
                                    XT[:, k * T + 128 : (k + 1) * T],
                                    XT[:, k * T + 128 : (k + 1) * T],
                                    start=(k == 0), stop=False,
                                )
                for gk in (NT // 2 - 2, NT // 2 - 1):
                    for b in pair:
                        XT = XTp[b]
                        for k in (2 * gk, 2 * gk + 1):
                            last = k == NT - 1
                            nc.tensor.matmul(
                                psc[(b, 0)][:, :T],
                                XT[:, k * T : k * T + 128],
                                XT[:, k * T : (k + 1) * T],
                                start=False, stop=last,
                            )
                            nc.tensor.matmul(
                                psc[(b, 1)][:, : T - 128],
                                XT[:, k * T + 128 : (k + 1) * T],
                                XT[:, k * T + 128 : (k + 1) * T],
                                start=False, stop=last,
                            )

                # --- AT = psc * decayT ---
                ATp = {}
                for b in pair:
                    AT = mid.tile([128, TT, T], mm_dt, tag="AT")
                    ATp[b] = AT
                    nc.vector.tensor_tensor(
                        out=AT[:, 0, :], in0=psc[(b, 0)][:, :T],
                        in1=decayT[:, 0, :], op=OP.mult,
                    )
                    nc.vector.tensor_tensor(
                        out=AT[:, 1, 128:], in0=psc[(b, 1)][:, :T - 128],
                        in1=decayT[:, 1, 128:], op=OP.mult,
                    )

                # --- a* = AT^T @ U, boosted; no LN (see header); -> aT ---
                pa = {}
                for tt in range(TT):
                    for b in pair:
                        pa[(b, tt)] = ps.tile([128, 512], F32, tag="psS", bufs=4,
                                              name=f"pa{b}_{tt}")
                        for k in range(tt + 1):
                            nc.tensor.matmul(
                                pa[(b, tt)][:, :D],
                                ATp[b][:, k, tt * 128 : (tt + 1) * 128],
                                u[:, b, k, :],
                                start=(k == 0), stop=(k == tt),
                            )
                aTp = {}
                for b in pair:
                    aTp[b] = mid.tile([128, DT, T], mm_dt, tag="aT", name=f"aT{b}")
                for tt in range(TT):
                    for i, b in enumerate(pair):
                        araw = scratch.tile([128, D], F32, tag="araw", bufs=2)
                        if i == 0:
                            nc.scalar.activation(out=araw[:], in_=pa[(b, tt)][:, :D],
                                                 func=AF.Copy, scale=A_BOOST)
                        else:
                            nc.vector.tensor_scalar(
                                out=araw[:], in0=pa[(b, tt)][:, :D],
                                scalar1=A_BOOST, scalar2=None, op0=OP.mult)
                        for kd in range(DT):
                            pt = ps.tile([128, 512], F32, tag="ps")
                            nc.tensor.transpose(
                                out=pt[:, :128],
                                in_=araw[:, kd * 128 : (kd + 1) * 128],
                                identity=ident[:],
                            )
                            nc.scalar.activation(
                                out=aTp[b][:, kd, tt * 128 : (tt + 1) * 128],
                                in_=pt[:, :128], func=AF.Copy,
                            )

                # --- y^T = relu(Dy @ a^T) * XT -> yq (aliases R), fused with
                # incremental v* = yq @ E^T accumulation ---
                pv = {}
                for tt in range(TT):
                    for b in pair:
                        pv[(b, tt)] = ps.tile([128, 512], F32, tag="psS", bufs=4,
                                              name=f"pv{b}_{tt}")

                def v_mms(gk, last):
                    for b2 in pair:
                        for tt2 in range(TT):
                            for k in (2 * gk, 2 * gk + 1):
                                nc.tensor.matmul(
                                    pv[(b2, tt2)][:, :D],
                                    Rb[b2][:, k * T + tt2 * 128 : k * T + tt2 * 128 + 128],
                                    et[:, k, :],
                                    start=(k == 0), stop=(last and k == NT - 1),
                                )

                for g in range(NT // 2):
                    for b in pair:
                        py = ps.tile([128, 512], F32, tag="ps")
                        for h in range(2):
                            nt = 2 * g + h
                            for kd in range(DT):
                                nc.tensor.matmul(
                                    py[:, T * h : T * h + T],
                                    dyt[:, kd, nt * 128 : (nt + 1) * 128],
                                    aTp[b][:, kd, :],
                                    start=(kd == 0), stop=(kd == DT - 1),
                                )
                        yq = Rb[b][:, 2 * g * T : (2 * g + 2) * T]
                        xin = XTp[b][:, 2 * g * T : (2 * g + 2) * T]
                        if g % 2 == 0:
                            yr = scratch.tile([128, 512], mm_dt, tag="yrelu", bufs=2)
                            nc.scalar.activation(out=yr[:], in_=py[:], func=AF.Relu)
                            nc.vector.tensor_tensor(
                                out=yq, in0=yr[:], in1=xin, op=OP.mult
                            )
                        else:
                            nc.vector.scalar_tensor_tensor(
                                out=yq, in0=py[:], scalar=0.0, in1=xin,
                                op0=OP.max, op1=OP.mult,
                            )
                    if g >= 2:
                        v_mms(g - 2, last=False)
                for gk in (NT // 2 - 2, NT // 2 - 1):
                    v_mms(gk, last=(gk == NT // 2 - 1))

                # --- v* = LN(v*_raw) -> out ---
                for tt in range(TT):
                    for i, b in enumerate(pair):
                        vstar = scratch.tile([128, D], F32, tag="vstar", bufs=2)
                        ln_rows(pv[(b, tt)][:, :D], vstar[:],
                                apply_on="scalar" if i == 0 else "vector")
                        nc.sync.dma_start(
                            out=out_d[b, tt * 128 : (tt + 1) * 128, :], in_=vstar[:]
                        )

    nc.compile()
    return nc


_NC_CACHE = {}


def _get_nc(mm_dt=MM_DT):
    key = str(mm_dt)
    if key not in _NC_CACHE:
        _NC_CACHE[key] = build_nc(mm_dt)
    return _NC_CACHE[key]


def prep_shared(token_emb, E, Dx, Dy):
    wdt = mybir.dt.np(MM_DT)
    return {
        "dxt": np.ascontiguousarray(np.asarray(Dx, np.float32).T.astype(wdt)),
        "dyt": np.ascontiguousarray(np.asarray(Dy, np.float32).T.astype(wdt)),
        "et": np.ascontiguousarray(np.asarray(E, np.float32).T.astype(wdt)),
        **_host_consts(),
    }


def prep_core(idx_core, token_emb):
    """Per-core embedding gather + LN + transpose (host-side input prep)."""
    wdt = mybir.dt.np(MM_DT)
    v = np.asarray(token_emb, np.float32)[np.asarray(idx_core)]   # (BL, T, D)
    m = v.mean(-1, keepdims=True)
    s = v.std(-1, keepdims=True, ddof=1)
    u = ((v - m) / (s + EPS)).astype(wdt)                         # LN rows
    # u layout [128, BL, TT, D]: partition = t within tile
    u = np.ascontiguousarray(
        u.reshape(BL, TT, 128, D).transpose(2, 0, 1, 3))
    # vpt layout [128, BL, DT, T]: partition = d within tile
    vpt = np.ascontiguousarray(
        v.transpose(0, 2, 1).reshape(BL, DT, 128, T)
        .transpose(2, 0, 1, 3).astype(wdt))
    return {"vpt": vpt, "u": u}


def kernel(idx, token_emb, E, Dx, Dy):
    idx = np.asarray(idx).astype(np.int64)
    nc = _get_nc()
    shared = prep_shared(token_emb, E, Dx, Dy)
    in_maps = []
    for c in range(NCORES):
        m = dict(shared)
        m.update(prep_core(idx[c * BL : (c + 1) * BL], token_emb))
        in_maps.append(m)

    res = run_bass_kernel_spmd(nc, in_maps, core_ids=list(range(NCORES)))
    out = np.concatenate([r["out"] for r in res.results], axis=0)
    return out
